# revision 2
# baseline (speedup 1.0000x reference)
"""AffinityPropagate Trainium2 kernel.

Reference computation (per batch element):
    k_d = softmax(guided_d, axis=channel)          d = 1,2,3 (dilations)
    repeat 8 times:
        o_d = sum_ch k_d[ch] * shift(x, offset(d, ch))
        x   = o_1*fuse[0] + o_2*fuse[1] + o_3*fuse[2]

Strategy: pure data parallel over the batch (8 batches -> 8 NeuronCores).
Per core, the three 9-tap dilated kernels are pre-fused with the fuse
weights into 25 distinct-offset weight fields (the three (0,0) taps
share one field) stored in fp16 in SBUF.  x is kept in a halo layout:
partition p owns image rows [4p, 4p+4), stored with 3 halo rows on each
side and 4 zero border columns on each side ([120, 10, 648] fp16).

Each iteration: for every tap, VectorE multiplies the weight field with
a shifted window of x (fp16, 2x perf mode); TensorE accumulates the 25
product fields into PSUM in fp32 via identity-stationary matmuls;
ScalarE evacuates PSUM back to the fp16 x buffer.  A one-element-shifted
copy of x (maintained by ScalarE) keeps odd column offsets 4B-aligned so
the DVE stays in 2x mode.  Halo rows are refreshed with two SBUF->SBUF
DMAs per iteration.
"""

import numpy as np

import concourse.bacc as bacc
import concourse.bass as bass
import concourse.mybir as mybir
from concourse.bass_utils import run_bass_kernel_spmd
from concourse.masks import make_identity
from concourse.tile import TileContext

H, W = 480, 640
P = 120          # partitions used (each owns R rows)
R = 4            # rows per partition
HALO = 3         # halo rows each side
CB = 4           # border cols each side (4 keeps packed reads 4B aligned)
ROWB = R + 2 * HALO          # 10 buffer rows per partition
COLB = W + 2 * CB            # 648 buffer cols
PROP_TIME = 8
NCORES = 8

F16 = mybir.dt.float16
F32 = mybir.dt.float32


def _tap_table():
    """field_of[(d_idx, ch)] -> weight-field index; taps: (field, dh, dw)."""
    field_of = {}
    taps = [(0, 0, 0)]          # field 0 = merged (0,0) center
    f = 1
    for d_idx, d in enumerate((1, 2, 3)):
        for ch in range(9):
            if ch == 4:
                field_of[(d_idx, ch)] = 0
                continue
            dh = (ch // 3 - 1) * d
            dw = (ch % 3 - 1) * d
            field_of[(d_idx, ch)] = f
            taps.append((f, dh, dw))
            f += 1
    assert f == 25
    return field_of, taps


FIELD_OF, TAPS = _tap_table()

# How many of the 25 tap-multiplies run on GPSIMD instead of DVE.
N_GPSIMD_MULTS = 0


def build_nc():
    nc = bacc.Bacc("TRN2", target_bir_lowering=False, debug=False)

    g_dram = [
        nc.dram_tensor(name, [9, H, W], F32, kind="ExternalInput")
        for name in ("guided1", "guided2", "guided3")
    ]
    fuse_dram = nc.dram_tensor("fuse", [3, H, W], F32, kind="ExternalInput")
    x_dram = nc.dram_tensor("x", [1, H, W], F32, kind="ExternalInput")
    out_dram = nc.dram_tensor("out", [1, H, W], F32, kind="ExternalOutput")

    # DRAM access patterns: partition p <- rows [4p, 4p+4)
    def rows_ap(t, extra_off=0):
        return bass.AP(t, extra_off, [[R * W, P], [W, R], [1, W]])

    with TileContext(nc) as tc:
        with (
            tc.tile_pool(name="const", bufs=1) as constp,
            tc.tile_pool(name="wpool", bufs=1) as wpool,
            tc.tile_pool(name="xmain", bufs=1) as xmain,
        ):
            ident = constp.tile([P, P], F16)
            make_identity(nc, ident)

            wt = [wpool.tile([P, R, W], F16, tag=f"w{t}", name=f"w{t}") for t in range(25)]
            XA = xmain.tile([P, ROWB, COLB], F16, tag="XA")

            # ---------------- setup: weights + x load ----------------
            with (
                tc.tile_pool(name="setup", bufs=2) as sp,
                tc.tile_pool(name="small", bufs=2) as smallp,
                tc.tile_pool(name="fusep", bufs=1) as fusep,
                tc.tile_pool(name="psst", bufs=2, space="PSUM") as psp,
            ):
                # x: load f32, convert to fp16 into the halo layout
                nc.vector.memset(XA, 0.0)
                xs32 = sp.tile([P, R, W], F32, tag="g")
                nc.sync.dma_start(out=xs32, in_=rows_ap(x_dram))
                nc.vector.tensor_copy(
                    out=XA[:, HALO:HALO + R, CB:CB + W], in_=xs32
                )
                # initial halo fill
                nc.sync.dma_start(
                    out=XA[1:P, 0:HALO, :], in_=XA[0:P - 1, R:R + HALO, :]
                )
                nc.sync.dma_start(
                    out=XA[0:P - 1, R + HALO:ROWB, :], in_=XA[1:P, HALO:2 * HALO, :]
                )

                CW = 320  # column chunk width (2 chunks per row-slot)
                for d_idx in range(3):
                    f16 = fusep.tile([P, R, W], F16, tag="f16")
                    f32t = sp.tile([P, R, W], F32, tag="g")
                    nc.sync.dma_start(
                        out=f32t, in_=rows_ap(fuse_dram, d_idx * H * W)
                    )
                    nc.vector.tensor_copy(out=f16, in_=f32t)
                    for j in range(R):
                        for h2 in range(2):
                            c0 = h2 * CW
                            g = sp.tile([P, 9, CW], F32, tag="g")
                            nc.sync.dma_start(
                                out=g,
                                in_=bass.AP(
                                    g_dram[d_idx],
                                    j * W + c0,
                                    [[R * W, P], [H * W, 9], [1, CW]],
                                ),
                            )
                            e = sp.tile([P, 9, CW], F16, tag="e")
                            nc.scalar.activation(
                                out=e, in_=g,
                                func=mybir.ActivationFunctionType.Exp,
                            )
                            ps = psp.tile([P, CW], F32)
                            for ch in range(9):
                                nc.tensor.matmul(
                                    out=ps, lhsT=ident, rhs=e[:, ch, :],
                                    start=(ch == 0), stop=(ch == 8),
                                )
                            r = smallp.tile([P, CW], F16, tag="r")
                            with nc.allow_low_precision("fp16 softmax recip"):
                                nc.vector.reciprocal(out=r, in_=ps)
                            t_ = smallp.tile([P, CW], F16, tag="t")
                            nc.vector.tensor_mul(
                                out=t_, in0=f16[:, j, c0:c0 + CW], in1=r
                            )
                            for ch in range(9):
                                fld = FIELD_OF[(d_idx, ch)]
                                wv = wt[fld][:, j, c0:c0 + CW]
                                if ch == 4 and d_idx > 0:
                                    tmp = smallp.tile([P, CW], F16, tag="tmp")
                                    nc.vector.tensor_mul(
                                        out=tmp, in0=e[:, ch, :], in1=t_
                                    )
                                    nc.vector.tensor_add(out=wv, in0=wv, in1=tmp)
                                else:
                                    nc.vector.tensor_mul(
                                        out=wv, in0=e[:, ch, :], in1=t_
                                    )

            # ---------------- iterations ----------------
            with (
                tc.tile_pool(name="xiter", bufs=1) as xiter,
                tc.tile_pool(name="mpool", bufs=4) as mpool,
                tc.tile_pool(name="stagep", bufs=1) as stagep,
                tc.tile_pool(name="psit", bufs=1, space="PSUM") as psi,
            ):
                XB = xiter.tile([P, ROWB, COLB], F16, tag="XB")
                XS = xiter.tile([P, ROWB, COLB], F16, tag="XS")
                nc.vector.memset(XB, 0.0)
                nc.vector.memset(XS, 0.0)

                bufs = [XA, XB]
                NFLAT = ROWB * COLB
                for it in range(PROP_TIME):
                    Xc = bufs[it % 2]
                    Xn = bufs[(it + 1) % 2]
                    Xc_f = Xc.rearrange("p a b -> p (a b)")
                    XS_f = XS.rearrange("p a b -> p (a b)")
                    # shifted copy: XS[k] = Xc[k+1]
                    nc.scalar.copy(
                        out=XS_f[:, 0:NFLAT - 1], in_=Xc_f[:, 1:NFLAT]
                    )

                    ps = psi.tile([P, R * W], F32)
                    n_dve = len(TAPS) - N_GPSIMD_MULTS
                    for t, (fld, dh, dw) in enumerate(TAPS):
                        if dw % 2 == 0:
                            src, coff = Xc, CB + dw
                        else:
                            src, coff = XS, CB - 1 + dw
                        m = mpool.tile([P, R, W], F16, tag="m")
                        eng = nc.vector if t < n_dve else nc.gpsimd
                        eng.tensor_mul(
                            out=m,
                            in0=wt[fld],
                            in1=src[:, HALO + dh:HALO + dh + R, coff:coff + W],
                        )
                        mf = m.rearrange("p a b -> p (a b)")
                        for k in range(5):
                            nc.tensor.matmul(
                                out=ps[:, k * 512:(k + 1) * 512],
                                lhsT=ident,
                                rhs=mf[:, k * 512:(k + 1) * 512],
                                start=(t == 0),
                                stop=(t == len(TAPS) - 1),
                            )
                    # evacuate PSUM -> Xn owned rows (f32 -> fp16)
                    nc.scalar.copy(
                        out=Xn[:, HALO:HALO + R, CB:CB + W],
                        in_=ps.rearrange("p (a b) -> p a b", a=R),
                    )
                    # halo refresh
                    nc.sync.dma_start(
                        out=Xn[1:P, 0:HALO, :], in_=Xn[0:P - 1, R:R + HALO, :]
                    )
                    nc.sync.dma_start(
                        out=Xn[0:P - 1, R + HALO:ROWB, :],
                        in_=Xn[1:P, HALO:2 * HALO, :],
                    )

                Xfin = bufs[PROP_TIME % 2]
                stage = stagep.tile([P, R, W], F32)
                nc.vector.tensor_copy(
                    out=stage, in_=Xfin[:, HALO:HALO + R, CB:CB + W]
                )
                nc.sync.dma_start(out=rows_ap(out_dram), in_=stage)

    nc.compile()
    return nc


_NC = None


def _get_nc():
    global _NC
    if _NC is None:
        _NC = build_nc()
    return _NC


def _in_maps(guided1, guided2, guided3, fuse, x):
    maps = []
    for b in range(NCORES):
        maps.append({
            "guided1": np.ascontiguousarray(guided1[b], dtype=np.float32),
            "guided2": np.ascontiguousarray(guided2[b], dtype=np.float32),
            "guided3": np.ascontiguousarray(guided3[b], dtype=np.float32),
            "fuse": np.ascontiguousarray(fuse[b], dtype=np.float32),
            "x": np.ascontiguousarray(x[b], dtype=np.float32),
        })
    return maps


def kernel(guided1, guided2, guided3, fuse, x):
    nc = _get_nc()
    res = run_bass_kernel_spmd(
        nc, _in_maps(guided1, guided2, guided3, fuse, x),
        core_ids=list(range(NCORES)),
    )
    return np.stack([res.results[b]["out"] for b in range(NCORES)], axis=0)


def kernel_profiled(guided1, guided2, guided3, fuse, x):
    """Returns (output, BassKernelResults) with trace enabled."""
    nc = _get_nc()
    res = run_bass_kernel_spmd(
        nc, _in_maps(guided1, guided2, guided3, fuse, x),
        core_ids=list(range(NCORES)), trace=True,
    )
    out = np.stack([res.results[b]["out"] for b in range(NCORES)], axis=0)
    return out, res
